# revision 5
# baseline (speedup 1.0000x reference)
"""Trainium2 Bass kernel for nn_DCT2D_Layer: 8x8 block 2D-DCT + zigzag feature map.

Input : img  [16, 3, 512, 512] f32
Output: feat [16, 192, 64, 64]  f32  where feat[b, c*64+k, ib, jb] is the
        k-th zigzag DCT coefficient of the 8x8 block (ib, jb) of channel c.

Strategy (per core; 8 cores, pure data parallel over the 48 (b,c) images):
  - For each 128x128 image tile X (rows (ib,h), cols (jb,w)):
      mm1: out1 = X.T @ R1   -> out1[(jb,w), (u,ib)]   (X is the stationary operand)
      mm2: out2 = out1.T @ R2 -> out2[(u,ib), (v,jb)]  (out1 is stationary)
    where R1[8*ib+h, 16*u+ib'] = C[u,h] * (ib==ib') and
          R2[8*jb+w, 16*v+jb'] = C[v,w] * (jb==jb') are 128x128 block-diagonal
    arrangements of the 8x8 DCT-II basis C.
  - out2 is copied PSUM->SBUF into a v-major staging layout
    [128 part=(u,ib), img, v, tI, tJ, jb], so that for each frequency pair
    (u,v) a single affine DMA writes the full 64x64 map of channel
    zig(u,v) for several images at once (dest is contiguous per channel).
"""

import numpy as np

import concourse.bacc as bacc
import concourse.bass as bass
import concourse.mybir as mybir
from concourse.tile import TileContext
from concourse.bass_utils import run_bass_kernel_spmd

N_CORES = 8
IMGS_TOTAL = 48          # 16 batches x 3 channels
IMGS_PER_CORE = IMGS_TOTAL // N_CORES   # 6
H = W = 512
B = 8                    # DCT block size
NT = 4                   # 128x128 tiles per image side


def _zigzag(n):
    idx = np.zeros(n * n, dtype=np.int64)
    i = j = 0
    for k in range(n * n):
        idx[k] = i * n + j
        if (i + j) % 2 == 0:
            if j == n - 1:
                i += 1
            elif i == 0:
                j += 1
            else:
                i -= 1
                j += 1
        else:
            if i == n - 1:
                j += 1
            elif j == 0:
                i += 1
            else:
                i += 1
                j -= 1
    return idx


def _dct_basis(N):
    k = np.arange(N)[:, None].astype(np.float64)
    nn = np.arange(N)[None, :].astype(np.float64)
    return (2.0 * np.cos(np.pi * (2.0 * nn + 1.0) * k / (2.0 * N))).astype(np.float32)


def _constants():
    C = _dct_basis(B)
    R1 = np.zeros((128, 128), np.float32)
    R2 = np.zeros((128, 128), np.float32)
    for blk in range(16):
        for x in range(8):
            for f in range(8):
                R1[8 * blk + x, 16 * f + blk] = C[f, x]
                R2[8 * blk + x, 16 * f + blk] = C[f, x]
    zz = _zigzag(B)
    ch_of_flat = np.empty(64, np.int64)
    ch_of_flat[zz] = np.arange(64)
    return R1, R2, ch_of_flat


R1_NP, R2_NP, CH_OF_FLAT = _constants()


def build_kernel(n_imgs=IMGS_PER_CORE):
    f32 = mybir.dt.float32
    nc = bacc.Bacc("TRN2", target_bir_lowering=False, debug=False,
                   num_devices=N_CORES)

    img = nc.dram_tensor("img", [n_imgs, H, W], f32, kind="ExternalInput")
    r1 = nc.dram_tensor("r1", [128, 128], f32, kind="ExternalInput")
    r2 = nc.dram_tensor("r2", [128, 128], f32, kind="ExternalInput")
    out = nc.dram_tensor("out", [n_imgs, 64, 64, 64], f32, kind="ExternalOutput")

    with TileContext(nc) as tc:
        with (
            tc.tile_pool(name="consts", bufs=1) as cpool,
            tc.tile_pool(name="strip", bufs=3) as spool,
            tc.tile_pool(name="o1", bufs=4) as o1pool,
            tc.tile_pool(name="outsb", bufs=2) as opool,
            tc.tile_pool(name="ps1", bufs=3, space="PSUM") as ps1,
            tc.tile_pool(name="ps2", bufs=3, space="PSUM") as ps2,
        ):
            r1t = cpool.tile([128, 128], f32)
            nc.sync.dma_start(out=r1t, in_=r1.ap())
            r2t = cpool.tile([128, 128], f32)
            nc.sync.dma_start(out=r2t, in_=r2.ap())

            tcount = 0
            for i in range(n_imgs):
                # staging layout: [part=(u,ib), v, tI, tJ, jb]
                outsb = opool.tile([128, 8, NT, NT, 16], f32)
                for tI in range(NT):
                    strip = spool.tile([128, W], f32)
                    nc.sync.dma_start(
                        out=strip, in_=img.ap()[i, 128 * tI:128 * (tI + 1), :]
                    )
                    for tJ in range(NT):
                        p1t = ps1.tile([128, 128], f32)
                        nc.tensor.matmul(
                            p1t, strip[:, 128 * tJ:128 * (tJ + 1)], r1t[:],
                            start=True, stop=True,
                        )
                        o1 = o1pool.tile([128, 128], f32)
                        if tcount % 3 != 0:
                            nc.vector.tensor_copy(out=o1[:], in_=p1t[:])
                        else:
                            nc.scalar.copy(out=o1[:], in_=p1t[:])

                        p2t = ps2.tile([128, 128], f32)
                        nc.tensor.matmul(
                            p2t, o1[:], r2t[:], start=True, stop=True,
                        )
                        dst2 = outsb[:, :, tI, tJ, :]
                        src2 = p2t[:].rearrange("p (v j) -> p v j", v=8)
                        if (tcount + 1) % 3 != 0:
                            nc.vector.tensor_copy(out=dst2, in_=src2)
                        else:
                            nc.scalar.copy(out=dst2, in_=src2)
                        tcount += 1
                # one DMA per frequency pair (u, v) for this image:
                # src [ib(16 part), tI(4), (tJ jb)=64 contiguous]
                # dst out[i, zig(u,v)] viewed as [ib, tI, w] (w = map row, 256B)
                for u in range(8):
                    for v in range(8):
                        k = int(CH_OF_FLAT[u * 8 + v])
                        src = outsb[16 * u:16 * (u + 1), v, :, :, :]
                        dst = out.ap()[i, k].rearrange(
                            "(ti ib) w -> ib ti w", ib=16
                        )
                        nc.sync.dma_start(out=dst, in_=src)

    nc.compile()
    return nc


_NC_CACHE = {}


def _get_nc(n_imgs):
    if n_imgs not in _NC_CACHE:
        _NC_CACHE[n_imgs] = build_kernel(n_imgs)
    return _NC_CACHE[n_imgs]


def run(img, trace=False):
    """img: [16,3,512,512] f32 -> (feat [16,192,64,64] f32, BassKernelResults)."""
    img = np.ascontiguousarray(np.asarray(img), dtype=np.float32)
    bs, c, h, w = img.shape
    flat = img.reshape(bs * c, h, w)
    nc = _get_nc(IMGS_PER_CORE)
    in_maps = [
        {
            "img": np.ascontiguousarray(flat[IMGS_PER_CORE * cid:
                                             IMGS_PER_CORE * (cid + 1)]),
            "r1": R1_NP,
            "r2": R2_NP,
        }
        for cid in range(N_CORES)
    ]
    res = run_bass_kernel_spmd(nc, in_maps, core_ids=list(range(N_CORES)),
                               trace=trace)
    shards = [res.results[cid]["out"] for cid in range(N_CORES)]
    feat = np.concatenate(shards, axis=0).reshape(bs, c * 64, 64, 64)
    return feat, res


def kernel(img):
    feat, _ = run(img, trace=False)
    return feat


# revision 8
# speedup vs baseline: 1.8610x; 1.8610x over previous
"""Trainium2 Bass kernel for nn_DCT2D_Layer: 8x8 block 2D-DCT + zigzag feature map.

Input : img  [16, 3, 512, 512] f32
Output: feat [16, 192, 64, 64]  f32  where feat[b, c*64+k, ib, jb] is the
        k-th zigzag DCT coefficient of the 8x8 block (ib, jb) of channel c.

Strategy (per core; 8 cores, pure data parallel over the 48 (b,c) images):
  - For each 128x128 image tile X (rows (ib,h), cols (jb,w)):
      mm1: out1 = X.T @ R1   -> out1[(jb,w), (u,ib)]   (X is the stationary operand)
      mm2: out2 = out1.T @ R2 -> out2[(u,ib), (v,jb)]  (out1 is stationary)
    where R1[8*ib+h, 16*u+ib'] = C[u,h] * (ib==ib') and
          R2[8*jb+w, 16*v+jb'] = C[v,w] * (jb==jb') are 128x128 block-diagonal
    arrangements of the 8x8 DCT-II basis C.
  - out2 is copied PSUM->SBUF into a v-major staging layout
    [128 part=(u,ib), img, v, tI, tJ, jb], so that for each frequency pair
    (u,v) a single affine DMA writes the full 64x64 map of channel
    zig(u,v) for several images at once (dest is contiguous per channel).
"""

import numpy as np

import concourse.bacc as bacc
import concourse.bass as bass
import concourse.mybir as mybir
from concourse.tile import TileContext
from concourse.bass_utils import run_bass_kernel_spmd

N_CORES = 8
IMGS_TOTAL = 48          # 16 batches x 3 channels
IMGS_PER_CORE = IMGS_TOTAL // N_CORES   # 6
H = W = 512
B = 8                    # DCT block size
NT = 4                   # 128x128 tiles per image side


def _zigzag(n):
    idx = np.zeros(n * n, dtype=np.int64)
    i = j = 0
    for k in range(n * n):
        idx[k] = i * n + j
        if (i + j) % 2 == 0:
            if j == n - 1:
                i += 1
            elif i == 0:
                j += 1
            else:
                i -= 1
                j += 1
        else:
            if i == n - 1:
                j += 1
            elif j == 0:
                i += 1
            else:
                i += 1
                j -= 1
    return idx


def _dct_basis(N):
    k = np.arange(N)[:, None].astype(np.float64)
    nn = np.arange(N)[None, :].astype(np.float64)
    return (2.0 * np.cos(np.pi * (2.0 * nn + 1.0) * k / (2.0 * N))).astype(np.float32)


def _constants():
    C = _dct_basis(B)
    # R1: out1 free / out2 partition arrangement n = 16*u + ib
    # (strided-partition DMA slices are not supported, so contiguous blocks)
    R1 = np.zeros((128, 128), np.float32)
    # R2: out2 free arrangement n = 16*v + jb (v-major for staging/stores)
    R2 = np.zeros((128, 128), np.float32)
    for blk in range(16):
        for x in range(8):
            for f in range(8):
                R1[8 * blk + x, 16 * f + blk] = C[f, x]
                R2[8 * blk + x, 16 * f + blk] = C[f, x]
    zz = _zigzag(B)
    ch_of_flat = np.empty(64, np.int64)
    ch_of_flat[zz] = np.arange(64)
    return R1, R2, ch_of_flat


R1_NP, R2_NP, CH_OF_FLAT = _constants()


def build_kernel(n_imgs=IMGS_PER_CORE):
    f32 = mybir.dt.float32
    nc = bacc.Bacc("TRN2", target_bir_lowering=False, debug=False,
                   num_devices=N_CORES)

    img = nc.dram_tensor("img", [n_imgs, H, W], f32, kind="ExternalInput")
    r1 = nc.dram_tensor("r1", [128, 128], f32, kind="ExternalInput")
    r2 = nc.dram_tensor("r2", [128, 128], f32, kind="ExternalInput")
    out = nc.dram_tensor("out", [n_imgs, 64, 64, 64], f32, kind="ExternalOutput")

    with TileContext(nc) as tc:
        with (
            tc.tile_pool(name="consts", bufs=1) as cpool,
            tc.tile_pool(name="strip", bufs=3) as spool,
            tc.tile_pool(name="o1", bufs=4) as o1pool,
            tc.tile_pool(name="outsb", bufs=2) as opool,
            tc.tile_pool(name="ps1", bufs=3, space="PSUM") as ps1,
            tc.tile_pool(name="ps2", bufs=3, space="PSUM") as ps2,
        ):
            r1t = cpool.tile([128, 128], f32)
            nc.gpsimd.dma_start(out=r1t, in_=r1.ap())
            r2t = cpool.tile([128, 128], f32)
            nc.gpsimd.dma_start(out=r2t, in_=r2.ap())

            tcount = 0
            store_n = 0
            for i in range(n_imgs):
                # staging layout: [part=(8*ib+u), v, tI, tJ, jb]
                outsb = opool.tile([128, 8, NT, NT, 16], f32)
                # whole image in one SWDGE DMA: [128 part, strip, col]
                imtile = spool.tile([128, NT, W], f32)
                nc.gpsimd.dma_start(
                    out=imtile,
                    in_=img.ap()[i].rearrange("(s p) w -> p s w", p=128),
                )
                for tI in range(NT):
                    for tJ in range(NT):
                        p1t = ps1.tile([128, 128], f32)
                        nc.tensor.matmul(
                            p1t, imtile[:, tI, 128 * tJ:128 * (tJ + 1)], r1t[:],
                            start=True, stop=True,
                        )
                        o1 = o1pool.tile([128, 128], f32)
                        if tcount % 4 != 0:
                            nc.vector.tensor_copy(out=o1[:], in_=p1t[:])
                        else:
                            nc.scalar.copy(out=o1[:], in_=p1t[:])

                        p2t = ps2.tile([128, 128], f32)
                        nc.tensor.matmul(
                            p2t, o1[:], r2t[:], start=True, stop=True,
                        )
                        dst2 = outsb[:, :, tI, tJ, :]
                        src2 = p2t[:].rearrange("p (v j) -> p v j", v=8)
                        if (tcount + 1) % 4 != 0:
                            nc.vector.tensor_copy(out=dst2, in_=src2)
                        else:
                            nc.scalar.copy(out=dst2, in_=src2)
                        tcount += 1
                # one DMA per frequency pair (u, v) for this image:
                # src [ib(16 part), tI(4), (tJ jb)=64 contiguous]
                # dst out[i, zig(u,v)] viewed as [ib, tI, w] (w = map row, 256B)
                for u in range(8):
                    for v in range(8):
                        k = int(CH_OF_FLAT[u * 8 + v])
                        src = outsb[16 * u:16 * (u + 1), v, :, :, :]
                        dst = out.ap()[i, k].rearrange(
                            "(ti ib) w -> ib ti w", ib=16
                        )
                        eng = (nc.sync, nc.scalar, nc.gpsimd)[store_n % 3]
                        eng.dma_start(out=dst, in_=src)
                        store_n += 1

    nc.compile()
    return nc


_NC_CACHE = {}


def _get_nc(n_imgs):
    if n_imgs not in _NC_CACHE:
        _NC_CACHE[n_imgs] = build_kernel(n_imgs)
    return _NC_CACHE[n_imgs]


def run(img, trace=False):
    """img: [16,3,512,512] f32 -> (feat [16,192,64,64] f32, BassKernelResults)."""
    img = np.ascontiguousarray(np.asarray(img), dtype=np.float32)
    bs, c, h, w = img.shape
    flat = img.reshape(bs * c, h, w)
    nc = _get_nc(IMGS_PER_CORE)
    in_maps = [
        {
            "img": np.ascontiguousarray(flat[IMGS_PER_CORE * cid:
                                             IMGS_PER_CORE * (cid + 1)]),
            "r1": R1_NP,
            "r2": R2_NP,
        }
        for cid in range(N_CORES)
    ]
    res = run_bass_kernel_spmd(nc, in_maps, core_ids=list(range(N_CORES)),
                               trace=trace)
    shards = [res.results[cid]["out"] for cid in range(N_CORES)]
    feat = np.concatenate(shards, axis=0).reshape(bs, c * 64, 64, 64)
    return feat, res


def kernel(img):
    feat, _ = run(img, trace=False)
    return feat
